# revision 21
# baseline (speedup 1.0000x reference)
"""Trainium2 Bass kernel for nn_DTDMN (dialog topic/discourse memory network).

Self-contained: takes FULL unsharded inputs, shards sentences (NS=1024) across
8 NeuronCores (128/core) data-parallel, runs one SPMD Bass/Tile program, and
gathers the full outputs. Only the final per-dialog last-index gather and the
[32,256]@[256,1] score matmul run on host (negligible work).

Device program per core (n=128 sentences), mostly in "transposed" layout
(feature dim on partitions, sentences on the free axis):
  - indirect-DMA gather of word embeddings + PE-transpose to [E, n]
  - 40-step GRU scan with gate tiles [128, 4, n]; biases folded into the
    matmuls via an appended ones-row (input side) / rank-1 PSUM accumulation
  - masked attention pooling: bulk tanh + v-reduction matmuls, flat softmax
    over t, weighted sum via PE-broadcast multiply + strided DVE reduce
  - bow encoders (xenc/ctxenc) with PE-transposed bow chunks, K=20000 accum
  - gumbel-softmax / gaussian samples using host-precomputed threefry noise
  - decoder (recon) as K=50 matmuls + rank-1 bias, streamed per 512 columns
  - NTM memory erase/update read algebraically reduced to
      outs = ms@mem - erase*(ms^2@mem) + upd*sum(ms^2)
"""

import math
from contextlib import ExitStack

import numpy as np

import concourse.bass as bass
import concourse.bacc as bacc
import concourse.mybir as mybir
import concourse.tile as tile
from concourse.bass import ds, ts
from concourse.bass_utils import run_bass_kernel_spmd
from concourse.masks import make_identity

F32 = mybir.dt.float32
I32 = mybir.dt.int32
AF = mybir.ActivationFunctionType
ALU = mybir.AluOpType
AX = mybir.AxisListType

V, E, H, D, K, M = 20000, 300, 512, 50, 50, 256
DS, NW = 5, 40
B, NS = 32, 1024
NCORES = 8
N = NS // NCORES          # 128 sentences per core
P = 128

VB = 512                  # bow contraction block (4 sub-chunks of 128)
N_VB = V // VB            # 39 full blocks
V_TAIL = V - N_VB * VB    # 32
DCB = 512                 # decoder output chunk
N_DCB = math.ceil(V / DCB)

_E_CHUNKS = [(0, 128), (128, 128), (256, 44)]   # E=300 split; last gets +1 bias row
DEBUG_TAPS = False
DEBUG_NO_SCAN = False


# --------------------------------------------------------------------------
# host-side threefry (bit-exact with jax) for the fixed sampling noise
# --------------------------------------------------------------------------

def _threefry_core(kp, x0, x1):
    rot = [[13, 15, 26, 6], [17, 29, 16, 24]]

    def rotl(x, amt):
        return ((x << np.uint32(amt)) | (x >> np.uint32(32 - amt))).astype(np.uint32)

    x = [x0.astype(np.uint32).copy(), x1.astype(np.uint32).copy()]
    ks = [np.uint32(kp[0]), np.uint32(kp[1]),
          np.uint32(np.uint32(kp[0]) ^ np.uint32(kp[1]) ^ np.uint32(0x1BD11BDA))]
    x[0] = (x[0] + ks[0]).astype(np.uint32)
    x[1] = (x[1] + ks[1]).astype(np.uint32)
    for i in range(5):
        for r in rot[i % 2]:
            x[0] = (x[0] + x[1]).astype(np.uint32)
            x[1] = rotl(x[1], r) ^ x[0]
        x[0] = (x[0] + ks[(i + 1) % 3]).astype(np.uint32)
        x[1] = (x[1] + ks[(i + 2) % 3] + np.uint32(i + 1)).astype(np.uint32)
    return x[0], x[1]


def _fold_in(key, data):
    # classic threefry_2x32 on the [hi, lo] pair of the folded data
    o0, o1 = _threefry_core(key, np.array([0], np.uint32), np.array([data], np.uint32))
    return np.array([o0[0], o1[0]], dtype=np.uint32)


def _uniform(key, shape, lo=0.0, hi=1.0):
    # jax threefry_partitionable path: counts = 64-bit iota as (hi, lo) pairs
    n = int(np.prod(shape))
    idx = np.arange(n, dtype=np.uint64)
    c1 = (idx >> np.uint64(32)).astype(np.uint32)
    c2 = (idx & np.uint64(0xFFFFFFFF)).astype(np.uint32)
    b1, b2 = _threefry_core(key, c1, c2)
    bits = (b1 ^ b2).reshape(shape)
    u = ((bits >> np.uint32(9)) | np.uint32(0x3F800000)).view(np.float32) - np.float32(1.0)
    u = u * np.float32(hi - lo) + np.float32(lo)
    return np.maximum(np.float32(lo), u)


def _erfinv(x):
    # Giles (2012) single-precision polynomials — matches XLA ErfInv.
    x = x.astype(np.float32)
    w = -np.log((np.float32(1.0) - x) * (np.float32(1.0) + x))
    ws = w - np.float32(2.5)
    p_small = np.zeros_like(x)
    for c in [2.81022636e-08, 3.43273939e-07, -3.5233877e-06, -4.39150654e-06,
              0.00021858087, -0.00125372503, -0.00417768164, 0.246640727,
              1.50140941]:
        p_small = p_small * ws + np.float32(c)
    wb = np.sqrt(np.maximum(w, np.float32(0.0))) - np.float32(3.0)
    p_big = np.zeros_like(x)
    for c in [-0.000200214257, 0.000100950558, 0.00134934322, -0.00367342844,
              0.00573950773, -0.0076224613, 0.00943887047, 1.00167406,
              2.83297682]:
        p_big = p_big * wb + np.float32(c)
    p = np.where(w < np.float32(5.0), p_small, p_big)
    return (p * x).astype(np.float32)


def _normal(key, shape):
    lo = np.nextafter(np.float32(-1.0), np.float32(0.0))
    u = _uniform(key, shape, lo=float(lo), hi=1.0)
    return (np.float32(np.sqrt(2.0)) * _erfinv(u)).astype(np.float32)


_NOISE = None


def _sampling_noise():
    # The reference computes its fixed sampling noise with jax.random at run
    # time. Threefry bit-streams differ between jax backends (the neuron
    # lowering of the rolled loop differs from CPU), so to match the reference
    # bit-for-bit we generate the noise with the same jax calls in-process.
    # Falls back to a numpy threefry (bit-exact with CPU jax) if jax is absent.
    global _NOISE
    if _NOISE is not None:
        return _NOISE
    try:
        import jax
        import jax.numpy as jnp

        rng = jax.random.key(42)
        u = jax.random.uniform(jax.random.fold_in(rng, 0), (DS, NS, D))
        gumbel = -jnp.log(-jnp.log(u + 1e-10) + 1e-10)
        eps = jax.random.normal(jax.random.fold_in(rng, 1), (NS, K))
        gumbel = np.asarray(gumbel, dtype=np.float32)
        eps = np.asarray(eps, dtype=np.float32)
    except Exception:
        key = np.array([0, 42], dtype=np.uint32)
        u = _uniform(_fold_in(key, 0), (DS, NS, D))
        gumbel = (-np.log(-np.log(u + np.float32(1e-10)) + np.float32(1e-10))).astype(np.float32)
        eps = _normal(_fold_in(key, 1), (NS, K))
    _NOISE = (gumbel, eps)
    return _NOISE


# --------------------------------------------------------------------------
# device program
# --------------------------------------------------------------------------

def _build_program():
    nc = bacc.Bacc("TRN2", target_bir_lowering=False, debug=False,
                   num_devices=NCORES)

    io = {}

    def inp(name, shape, dtype=F32):
        io[name] = nc.dram_tensor(name, shape, dtype, kind="ExternalInput")
        return io[name]

    d_wid = inp("wid", [N, NW], I32)
    d_bow = inp("bow", [N, V])
    d_emb = inp("emb_tab", [V, E])
    d_wihT = inp("wihT", [P, 3, 3 * H])
    d_whhT = inp("whhT", [P, 4, 3 * H])
    d_bhn = inp("b_hn_row", [1, H])
    d_attnWT = inp("attnWT", [P, 4, H])
    d_attnb = inp("attn_b_col", [P, 4])
    d_attnv = inp("attn_v_col", [P, 4])
    d_addmask = inp("addmask", [1, NW * N])
    d_xencWT = inp("xencWT", [V, D])
    d_xencb = inp("xenc_b_row", [1, D])
    d_ctxencWT = inp("ctxencWT", [V, H])
    d_ctxencb = inp("ctxenc_b_row", [1, H])
    d_ctxencb_blk = inp("ctxenc_b_blk", [4, P])
    d_indicator = inp("indicator", [4, 4 * P])
    d_zmuWT = inp("zmuWT", [P, 4, D])
    d_zmub = inp("zmu_b_row", [1, D])
    d_zlvWT = inp("zlvWT", [P, 4, D])
    d_zlvb = inp("zlv_b_row", [1, D])
    d_gumbel = inp("gumbel", [N, DS * D])
    d_epsT = inp("epsT", [K, N])
    d_xdecWT = inp("xdecWT", [D, V])
    d_ctxdecWT = inp("ctxdecWT", [K, V])
    d_decb = inp("dec_b_row", [1, V])
    d_eraserWT = inp("eraserWT", [P, 4, M])
    d_eraserb = inp("eraser_b_row", [1, M])
    d_updateWT = inp("updateWT", [P, 4, M])
    d_updateb = inp("update_b_row", [1, M])
    d_mem = inp("memW", [D + K, M])

    d_recon = nc.dram_tensor("recon", [N, V], F32, kind="ExternalOutput")
    d_outsT = nc.dram_tensor("outsT", [2, P, N], F32, kind="ExternalOutput")
    taps = {}
    if DEBUG_TAPS:
        for nm, shp in [("t_qd", [P, D]), ("t_ctxT", [P, 4, N]), ("t_sdT", [D, N]),
                        ("t_stT", [K, N]), ("t_softtT", [K, N]), ("t_sembT", [P, 4, N]),
                        ("t_hlast", [P, 4, N]), ("t_er", [P, 2, N]), ("t_up", [P, 2, N])]:
            taps[nm] = nc.dram_tensor(nm, shp, F32, kind="ExternalOutput")

    with tile.TileContext(nc) as tc, ExitStack() as ctx:
        # ------------- pools (PSUM: 8 banks total in flight) -------------
        const_p = ctx.enter_context(tc.tile_pool(name="const", bufs=1))
        hall_p = ctx.enter_context(tc.tile_pool(name="hall", bufs=1))
        xp_ps = ctx.enter_context(tc.tile_pool(name="xp_ps", bufs=1, space="PSUM"))
        gate_ps = ctx.enter_context(tc.tile_pool(name="gate_ps", bufs=1, space="PSUM"))
        small_ps = ctx.enter_context(tc.tile_pool(name="small_ps", bufs=1, space="PSUM"))

        # ------------- constants / small persistent inputs -------------
        ident = const_p.tile([P, P], F32)
        make_identity(nc, ident[:])
        ones_n = const_p.tile([1, P], F32)
        nc.vector.memset(ones_n[:], 1.0)
        ones_col = const_p.tile([P, 1], F32)
        nc.vector.memset(ones_col[:], 1.0)

        wid_sb = const_p.tile([P, NW], I32)
        nc.sync.dma_start(wid_sb[:], d_wid[:, :])
        bhn_sb = const_p.tile([1, H], F32)
        nc.sync.dma_start(bhn_sb[:], d_bhn[:, :])
        attnb_sb = const_p.tile([P, 4], F32)
        nc.sync.dma_start(attnb_sb[:], d_attnb[:, :])
        attnv_sb = const_p.tile([P, 4], F32)
        nc.sync.dma_start(attnv_sb[:], d_attnv[:, :])
        addmask_sb = const_p.tile([1, NW * N], F32)
        nc.sync.dma_start(addmask_sb[:], d_addmask[:, :])
        xencb_sb = const_p.tile([1, D], F32)
        nc.sync.dma_start(xencb_sb[:], d_xencb[:, :])
        ctxencb_sb = const_p.tile([1, H], F32)
        nc.sync.dma_start(ctxencb_sb[:], d_ctxencb[:, :])
        ctxencb_blk_sb = const_p.tile([4, P], F32)
        nc.sync.dma_start(ctxencb_blk_sb[:], d_ctxencb_blk[:, :])
        ind_sb = const_p.tile([4, 4 * P], F32)
        nc.sync.dma_start(ind_sb[:], d_indicator[:, :])
        zmuWT_sb = const_p.tile([P, 4, D], F32)
        nc.sync.dma_start(zmuWT_sb[:], d_zmuWT[:, :, :])
        zmub_sb = const_p.tile([1, D], F32)
        nc.sync.dma_start(zmub_sb[:], d_zmub[:, :])
        zlvWT_sb = const_p.tile([P, 4, D], F32)
        nc.sync.dma_start(zlvWT_sb[:], d_zlvWT[:, :, :])
        zlvb_sb = const_p.tile([1, D], F32)
        nc.sync.dma_start(zlvb_sb[:], d_zlvb[:, :])
        gum_sb = const_p.tile([P, DS * D], F32)
        nc.sync.dma_start(gum_sb[:], d_gumbel[:, :])
        epsT_sb = const_p.tile([K, N], F32)
        nc.sync.dma_start(epsT_sb[:], d_epsT[:, :])
        mem_d = const_p.tile([D, M], F32)
        nc.sync.dma_start(mem_d[:, :], d_mem[0:D, :])
        mem_t = const_p.tile([K, M], F32)
        nc.sync.dma_start(mem_t[:, :], d_mem[D:D + K, :])
        # persistent cross-phase activations
        sdT = const_p.tile([D, N], F32)       # sample_d.T
        stT = const_p.tile([K, N], F32)       # sample_t.T (pre-softmax)
        softtT = const_p.tile([K, N], F32)    # softmax(sample_t).T
        sembT = const_p.tile([P, 4, N], F32)  # sent_emb.T k-chunks
        ctxT = const_p.tile([P, 4, N], F32)   # tanh ctx encoder out, k-chunks

        # dummy PE read of ident so later transposes don't double-wait on it
        ps_warm = xp_ps.tile([P, VB], F32, tag="xp", name="ps_warm")
        nc.tensor.transpose(ps_warm[:, :P], ident[:], ident[:])

        # embT double buffers; row 44 of chunk 2 = persistent ones (bias row)
        embT_a = const_p.tile([P, 3, P], F32)
        embT_b = const_p.tile([P, 3, P], F32)
        nc.sync.dma_start(embT_a[44:45, 2, :], ones_n[0:1, :])
        nc.sync.dma_start(embT_b[44:45, 2, :], ones_n[0:1, :])

        # h_all: [128, 4 k-chunks, (NW+1)*N]; block 0 = h_{-1} = 0
        hall = hall_p.tile([P, 4, (NW + 1) * N], F32)
        nc.gpsimd.memset(hall[:, :, 0:N], 0.0)

        # =================================================================
        # Phase B: GRU scan
        # =================================================================
        with tc.tile_pool(name="wscan", bufs=1) as wscan_p, \
             tc.tile_pool(name="scan_sb", bufs=2) as scan_io, \
             tc.tile_pool(name="gates_sb", bufs=1) as gsb:
            wihT_sb = wscan_p.tile([P, 3, 3 * H], F32)
            nc.sync.dma_start(wihT_sb[:], d_wihT[:, :, :])
            whhT_sb = wscan_p.tile([P, 4, 3 * H], F32)
            nc.sync.dma_start(whhT_sb[:], d_whhT[:, :, :])
            for t in range(NW if not DEBUG_NO_SCAN else 0):
                hb = t * N            # read offset in hall free dim
                wb_ = (t + 1) * N     # write offset

                embG = scan_io.tile([P, 304], F32, tag="embG")
                nc.gpsimd.indirect_dma_start(
                    out=embG[:, 0:E],
                    out_offset=None,
                    in_=d_emb[:, :],
                    in_offset=bass.IndirectOffsetOnAxis(ap=wid_sb[:, t:t + 1], axis=0),
                )
                ps_x = xp_ps.tile([P, VB], F32, tag="xp")
                nc.tensor.matmul(ps_x[0:1, 0:1], ones_n[0:1, 0:1], ones_n[0:1, 0:1],
                                 start=True, stop=True)
                for c, (c0, cw) in enumerate(_E_CHUNKS):
                    nc.tensor.transpose(ps_x[:cw, ts(c, P)], embG[:, ds(c0, cw)], ident[:])
                embT = embT_a if t % 2 == 0 else embT_b
                nc.scalar.copy(embT[:, 0:2, :], ps_x[:, 0:2 * P].rearrange("p (c n) -> p c n", c=2))
                nc.scalar.copy(embT[0:44, 2, :], ps_x[0:44, ts(2, P)])

                # gate matmuls: r (cols 0:512), z (512:1024), n-input (1024:1536)
                ps_r = gate_ps.tile([P, 4, N], F32, tag="g_r")
                ps_z = gate_ps.tile([P, 4, N], F32, tag="g_z")
                ps_in = gate_ps.tile([P, 4, N], F32, tag="g_in")
                ps_hn = gate_ps.tile([P, 4, N], F32, tag="g_hn")
                for m in range(4):
                    for gi, (ps_g, col0) in enumerate([(ps_r, 0), (ps_z, H), (ps_in, 2 * H)]):
                        for c, (c0, cw) in enumerate(_E_CHUNKS):
                            kw = cw + (1 if c == 2 else 0)  # bias row rides chunk 2
                            nc.tensor.matmul(
                                ps_g[:, m, :],
                                wihT_sb[:kw, c, ds(col0 + m * P, P)],
                                embT[:kw, c, :],
                                start=(c == 0),
                                stop=(c == 2 and ps_g is ps_in),
                            )
                        if ps_g is not ps_in:
                            for kc in range(4):
                                nc.tensor.matmul(
                                    ps_g[:, m, :],
                                    whhT_sb[:, kc, ds(col0 + m * P, P)],
                                    hall[:, kc, ds(hb, N)],
                                    start=False,
                                    stop=(kc == 3),
                                )
                    for kc in range(4):
                        nc.tensor.matmul(
                            ps_hn[:, m, :],
                            whhT_sb[:, kc, ds(2 * H + m * P, P)],
                            hall[:, kc, ds(hb, N)],
                            start=(kc == 0),
                            stop=False,
                        )
                    nc.tensor.matmul(
                        ps_hn[:, m, :], bhn_sb[0:1, ts(m, P)], ones_n[:],
                        start=False, stop=True,
                    )

                r_sb = gsb.tile([P, 4, N], F32, tag="r")
                nc.scalar.activation(r_sb[:], ps_r[:], AF.Sigmoid)
                z_sb = gsb.tile([P, 4, N], F32, tag="z")
                nc.scalar.activation(z_sb[:], ps_z[:], AF.Sigmoid)
                rn_sb = gsb.tile([P, 4, N], F32, tag="rn")
                nc.vector.tensor_tensor(rn_sb[:], ps_hn[:], r_sb[:], op=ALU.mult)
                s_sb = gsb.tile([P, 4, N], F32, tag="s")
                nc.vector.tensor_tensor(s_sb[:], ps_in[:], rn_sb[:], op=ALU.add)
                n_sb = gsb.tile([P, 4, N], F32, tag="n")
                nc.scalar.activation(n_sb[:], s_sb[:], AF.Tanh)
                d_sb = gsb.tile([P, 4, N], F32, tag="d")
                nc.gpsimd.tensor_tensor(d_sb[:], hall[:, :, ds(hb, N)], n_sb[:], op=ALU.subtract)
                e_sb = gsb.tile([P, 4, N], F32, tag="e")
                nc.gpsimd.tensor_tensor(e_sb[:], z_sb[:], d_sb[:], op=ALU.mult)
                nc.vector.tensor_tensor(hall[:, :, ds(wb_, N)], e_sb[:], n_sb[:], op=ALU.add)

        # =================================================================
        # Phase C: bow encoders (xenc + ctxenc), gumbel draw, sample_t
        # =================================================================
        with tc.tile_pool(name="enc_ps", bufs=1, space="PSUM") as enc_ps, \
             tc.tile_pool(name="enc_io", bufs=2) as enc_io:
            ps_ctx = enc_ps.tile([P, 4, N], F32, tag="ctx")
            ps_qd = small_ps.tile([D, N], F32, tag="sm", name="ps_qd")
            # ONE bank-wide start: bias-block lhsT [4,128] x indicator [4,512]
            # (start=True clears has_written for the WHOLE bank, so per-quarter
            # starts would erase each other's bias)
            nc.tensor.matmul(ps_ctx[:, :, :], ctxencb_blk_sb[:, :], ind_sb[:, :],
                             start=True, stop=False)
            nc.tensor.matmul(ps_qd[:, :], xencb_sb[0:1, :], ones_n[:],
                             start=True, stop=False)

            for kb in range(N_VB + 1):
                k0 = kb * VB
                kw = VB if kb < N_VB else V_TAIL
                nsub = kw // P if kw >= P else 1
                subw = P if kw >= P else kw
                last_kb = kb == N_VB
                bw = enc_io.tile([P, VB], F32, tag="bow")
                nc.sync.dma_start(bw[:, :kw], d_bow[:, ds(k0, kw)])
                ps_x = xp_ps.tile([P, VB], F32, tag="xp")
                nc.tensor.matmul(ps_x[0:1, 0:1], ones_n[0:1, 0:1], ones_n[0:1, 0:1],
                                 start=True, stop=True)
                for q in range(nsub):
                    nc.tensor.transpose(ps_x[:subw, ts(q, P)], bw[:, ds(q * subw, subw)], ident[:])
                bT = enc_io.tile([P, VB], F32, tag="bT")
                nc.scalar.copy(bT[:subw, : nsub * P], ps_x[:subw, : nsub * P])
                wct = enc_io.tile([P, 4, H], F32, tag="wct")
                nc.sync.dma_start(
                    wct[:subw, :nsub, :],
                    d_ctxencWT[ds(k0, kw), :].rearrange("(q p) m -> p q m", p=subw),
                )
                xw = enc_io.tile([P, 4, D], F32, tag="xw")
                nc.sync.dma_start(
                    xw[:subw, :nsub, :],
                    d_xencWT[ds(k0, kw), :].rearrange("(q p) m -> p q m", p=subw),
                )
                for q in range(nsub):
                    last = last_kb and q == nsub - 1
                    for m in range(4):
                        nc.tensor.matmul(
                            ps_ctx[:, m, :], wct[:subw, q, ts(m, P)], bT[:subw, ts(q, P)],
                            start=False, stop=last,
                        )
                    nc.tensor.matmul(
                        ps_qd[:, :], xw[:subw, q, :], bT[:subw, ts(q, P)],
                        start=False, stop=last,
                    )

            nc.scalar.activation(ctxT[:], ps_ctx[:], AF.Tanh)
            if DEBUG_TAPS:
                nc.sync.dma_start(taps["t_ctxT"][:, :, :], ctxT[:])

            # ---- gumbel-softmax sample_d (natural layout [n, 50]) ----
            ps_sm = small_ps.tile([P, VB], F32, tag="sm")
            qdT_sb = enc_io.tile([P, N], F32, tag="t1")
            nc.vector.tensor_copy(qdT_sb[:D, :], ps_qd[:, :])
            ps_sm2 = small_ps.tile([P, VB], F32, tag="sm")
            nc.tensor.transpose(ps_sm2[:, :D], qdT_sb[:D, :], ident[:D, :D])
            qd_sb = enc_io.tile([P, N], F32, tag="t2")
            nc.vector.tensor_copy(qd_sb[:, :D], ps_sm2[:, :D])
            if DEBUG_TAPS:
                nc.sync.dma_start(taps["t_qd"][:, :], qd_sb[:, :D])

            acc = [
                enc_io.tile([P, D], F32, tag="acc0", name="acc0"),
                enc_io.tile([P, D], F32, tag="acc1", name="acc1"),
            ]
            for s in range(DS):
                x = enc_io.tile([P, D], F32, tag="gx")
                nc.vector.tensor_tensor(x[:], qd_sb[:, :D], gum_sb[:, ds(s * D, D)], op=ALU.add)
                mx = enc_io.tile([P, 1], F32, tag="gmx")
                nc.vector.tensor_reduce(mx[:], x[:], axis=AX.X, op=ALU.max, negate=True)
                ex = enc_io.tile([P, D], F32, tag="gex")
                sume = enc_io.tile([P, 1], F32, tag="gsum")
                nc.scalar.activation(ex[:], x[:], AF.Exp, bias=mx[:, 0:1], accum_out=sume[:, 0:1])
                rs = enc_io.tile([P, 1], F32, tag="grs")
                nc.vector.reciprocal(rs[:], sume[:])
                if s == 0:
                    nc.vector.tensor_scalar_mul(acc[0][:], ex[:], rs[:, 0:1])
                else:
                    nc.vector.scalar_tensor_tensor(
                        acc[s % 2][:], ex[:], rs[:, 0:1], acc[(s + 1) % 2][:],
                        op0=ALU.mult, op1=ALU.add,
                    )
            sd_sb = enc_io.tile([P, D], F32, tag="sd")
            nc.scalar.mul(sd_sb[:], acc[(DS - 1) % 2][:], 1.0 / DS)
            ps_sd = small_ps.tile([P, VB], F32, tag="sm")
            nc.tensor.transpose(ps_sd[:D, :N], sd_sb[:], ident[:])
            nc.vector.tensor_copy(sdT[:, :], ps_sd[:D, :N])
            if DEBUG_TAPS:
                nc.sync.dma_start(taps["t_sdT"][:, :], sdT[:])

            # ---- gaussian sample_t (transposed [50, n]) ----
            ps_zlv = small_ps.tile([P, VB], F32, tag="sm")
            nc.tensor.matmul(ps_zlv[:D, :N], zlvb_sb[0:1, :], ones_n[:], start=True, stop=False)
            for kc in range(4):
                nc.tensor.matmul(ps_zlv[:D, :N], zlvWT_sb[:, kc, :], ctxT[:, kc, :],
                                 start=False, stop=(kc == 3))
            exl = enc_io.tile([K, N], F32, tag="exl")
            nc.scalar.activation(exl[:], ps_zlv[:D, :N], AF.Exp, scale=0.5)
            tm = enc_io.tile([K, N], F32, tag="tm")
            nc.vector.tensor_tensor(tm[:], exl[:], epsT_sb[:], op=ALU.mult)
            ps_zmu = small_ps.tile([P, VB], F32, tag="sm")
            nc.tensor.matmul(ps_zmu[:D, :N], zmub_sb[0:1, :], ones_n[:], start=True, stop=False)
            for kc in range(4):
                nc.tensor.matmul(ps_zmu[:D, :N], zmuWT_sb[:, kc, :], ctxT[:, kc, :],
                                 start=False, stop=(kc == 3))
            nc.vector.tensor_tensor(stT[:, :], tm[:], ps_zmu[:D, :N], op=ALU.add)
            if DEBUG_TAPS:
                nc.sync.dma_start(taps["t_stT"][:, :], stT[:])

            # softmax(sample_t) over K (via transpose to [n, 50] and back)
            ps_st = small_ps.tile([P, VB], F32, tag="sm")
            nc.tensor.transpose(ps_st[:, :K], stT[:, :], ident[:K, :K])
            st_sb = enc_io.tile([P, K], F32, tag="st")
            nc.vector.tensor_copy(st_sb[:], ps_st[:, :K])
            mx2 = enc_io.tile([P, 1], F32, tag="gmx")
            nc.vector.tensor_reduce(mx2[:], st_sb[:], axis=AX.X, op=ALU.max, negate=True)
            ex2 = enc_io.tile([P, K], F32, tag="gex2")
            sume2 = enc_io.tile([P, 1], F32, tag="gsum")
            nc.scalar.activation(ex2[:], st_sb[:], AF.Exp, bias=mx2[:, 0:1], accum_out=sume2[:, 0:1])
            rs2 = enc_io.tile([P, 1], F32, tag="grs")
            nc.vector.reciprocal(rs2[:], sume2[:])
            sfm = enc_io.tile([P, K], F32, tag="sfm")
            nc.vector.tensor_scalar_mul(sfm[:], ex2[:], rs2[:, 0:1])
            ps_st2 = small_ps.tile([P, VB], F32, tag="sm")
            nc.tensor.transpose(ps_st2[:K, :N], sfm[:], ident[:])
            nc.vector.tensor_copy(softtT[:, :], ps_st2[:K, :N])
            if DEBUG_TAPS:
                nc.sync.dma_start(taps["t_softtT"][:, :], softtT[:])

        # =================================================================
        # Phase D: attention pooling -> sembT (streamed per 512-col chunk)
        # =================================================================
        NB = NW * N // VB   # 10 chunks; chunk nb covers t = 4*nb .. 4*nb+3
        with tc.tile_pool(name="attn_w", bufs=1) as attn_w, \
             tc.tile_pool(name="attn_sb", bufs=1) as attn_sb, \
             tc.tile_pool(name="prod_sb", bufs=2) as prod_p, \
             tc.tile_pool(name="u_sb", bufs=2) as u_p:
            attnWT_sb = attn_w.tile([P, 4, H], F32)
            nc.sync.dma_start(attnWT_sb[:], d_attnWT[:, :, :])
            den_parts = attn_sb.tile([1, NB * N], F32, tag="denp")
            num_parts = [
                attn_sb.tile([P, NB, N], F32, tag=f"np{m}", name=f"np{m}")
                for m in range(4)
            ]
            for nb in range(NB):
                u_sb = u_p.tile([P, 4, VB], F32, tag="u")
                for m in range(4):
                    ps_u = gate_ps.tile([P, VB], F32, tag=["g_r", "g_z", "g_in", "g_hn"][m],
                                        name="ps_u")
                    for kc in range(4):
                        nc.tensor.matmul(
                            ps_u[:, :], attnWT_sb[:, kc, ts(m, P)],
                            hall[:, kc, ds(N + nb * VB, VB)],
                            start=(kc == 0), stop=(kc == 3),
                        )
                    nc.scalar.activation(u_sb[:, m, :], ps_u[:], AF.Tanh,
                                         bias=attnb_sb[:, m:m + 1])
                ps_s = small_ps.tile([P, VB], F32, tag="sm", name="ps_s")
                for m in range(4):
                    nc.tensor.matmul(ps_s[0:1, :], attnv_sb[:, m:m + 1], u_sb[:, m, :],
                                     start=(m == 0), stop=(m == 3))
                sraw = attn_sb.tile([1, VB], F32, tag="sraw")
                nc.vector.tensor_tensor(sraw[:], ps_s[0:1, :],
                                        addmask_sb[0:1, ds(nb * VB, VB)], op=ALU.add)
                p_nb = attn_sb.tile([1, VB], F32, tag="pnb")
                nc.scalar.activation(p_nb[:], sraw[:], AF.Exp)
                nc.vector.tensor_reduce(
                    den_parts[0:1, ds(nb * N, N)],
                    p_nb[:].rearrange("a (t i) -> a i t", i=N),
                    axis=AX.X, op=ALU.add,
                )
                ps_pb = xp_ps.tile([P, VB], F32, tag="xp", name="ps_pb")
                nc.tensor.matmul(ps_pb[:], ones_n[:], p_nb[:], start=True, stop=True)
                pb_sb = prod_p.tile([P, VB], F32, tag="pb")
                nc.scalar.copy(pb_sb[:], ps_pb[:])
                for m in range(4):
                    prod = prod_p.tile([P, VB], F32, tag="prod")
                    nc.vector.tensor_tensor(prod[:], hall[:, m, ds(N + nb * VB, VB)],
                                            pb_sb[:], op=ALU.mult)
                    nc.vector.tensor_reduce(
                        num_parts[m][:, nb, :],
                        prod[:].rearrange("p (t i) -> p i t", i=N),
                        axis=AX.X, op=ALU.add,
                    )

            den = attn_sb.tile([1, N], F32, tag="den")
            nc.vector.tensor_reduce(
                den[:], den_parts[:].rearrange("a (nb i) -> a i nb", i=N),
                axis=AX.X, op=ALU.add,
            )
            rden = attn_sb.tile([1, N], F32, tag="rden")
            nc.vector.reciprocal(rden[:], den[:])
            ps_rb = small_ps.tile([P, VB], F32, tag="sm", name="ps_rb")
            nc.tensor.matmul(ps_rb[:, :N], ones_n[:], rden[:], start=True, stop=True)
            rb_sb = attn_sb.tile([P, N], F32, tag="rb")
            nc.vector.tensor_copy(rb_sb[:], ps_rb[:, :N])
            if DEBUG_TAPS:
                nc.sync.dma_start(taps["t_hlast"][:, :, :], hall[:, :, ds(NW * N, N)])
            for m in range(4):
                numt = attn_sb.tile([P, N], F32, tag="numt")
                nc.vector.tensor_reduce(
                    numt[:], num_parts[m][:].rearrange("p nb i -> p i nb"),
                    axis=AX.X, op=ALU.add,
                )
                nc.vector.tensor_tensor(sembT[:, m, :], numt[:], rb_sb[:], op=ALU.mult)
            if DEBUG_TAPS:
                nc.sync.dma_start(taps["t_sembT"][:, :, :], sembT[:])

        # =================================================================
        # Phase E: erase/update memory read -> outsT
        # =================================================================
        with tc.tile_pool(name="mem_sb", bufs=1) as memp:
            eraserWT_sb = memp.tile([P, 4, M], F32)
            nc.sync.dma_start(eraserWT_sb[:], d_eraserWT[:, :, :])
            updateWT_sb = memp.tile([P, 4, M], F32)
            nc.sync.dma_start(updateWT_sb[:], d_updateWT[:, :, :])
            eraserb_sb = memp.tile([1, M], F32)
            nc.sync.dma_start(eraserb_sb[:], d_eraserb[:, :])
            updateb_sb = memp.tile([1, M], F32)
            nc.sync.dma_start(updateb_sb[:], d_updateb[:, :])

            er_sb = memp.tile([P, 2, N], F32)
            up_sb = memp.tile([P, 2, N], F32)
            for w_sb, b_sb, fn, dst in [
                (eraserWT_sb, eraserb_sb, AF.Sigmoid, er_sb),
                (updateWT_sb, updateb_sb, AF.Tanh, up_sb),
            ]:
                for m in range(2):
                    ps_eu = small_ps.tile([P, VB], F32, tag="sm")
                    nc.tensor.matmul(ps_eu[:, :N], b_sb[0:1, ts(m, P)], ones_n[:],
                                     start=True, stop=False)
                    for kc in range(4):
                        nc.tensor.matmul(ps_eu[:, :N], w_sb[:, kc, ts(m, P)],
                                         sembT[:, kc, :], start=False, stop=(kc == 3))
                    nc.scalar.activation(dst[:, m, :], ps_eu[:, :N], fn)
            if DEBUG_TAPS:
                nc.sync.dma_start(taps["t_er"][:, :, :], er_sb[:])
                nc.sync.dma_start(taps["t_up"][:, :, :], up_sb[:])

            sd2 = memp.tile([D, N], F32)
            nc.vector.tensor_tensor(sd2[:], sdT[:], sdT[:], op=ALU.mult)
            st2 = memp.tile([K, N], F32)
            nc.vector.tensor_tensor(st2[:], stT[:], stT[:], op=ALU.mult)
            ps_t12 = small_ps.tile([P, VB], F32, tag="t12")
            for m in range(2):
                nc.tensor.matmul(ps_t12[:, ds(m * N, N)], mem_d[:, ts(m, P)],
                                 sdT[:, :], start=True, stop=False)
                nc.tensor.matmul(ps_t12[:, ds(m * N, N)], mem_t[:, ts(m, P)],
                                 stT[:, :], start=False, stop=True)
            for m in range(2):
                nc.tensor.matmul(ps_t12[:, ds((2 + m) * N, N)], mem_d[:, ts(m, P)],
                                 sd2[:, :], start=True, stop=False)
                nc.tensor.matmul(ps_t12[:, ds((2 + m) * N, N)], mem_t[:, ts(m, P)],
                                 st2[:, :], start=False, stop=True)
            ps_s2 = small_ps.tile([P, VB], F32, tag="sm")
            nc.tensor.matmul(ps_s2[0:1, :N], ones_col[:D, :], sd2[:, :],
                             start=True, stop=False)
            nc.tensor.matmul(ps_s2[0:1, :N], ones_col[:K, :], st2[:, :],
                             start=False, stop=True)
            s2_sb = memp.tile([1, N], F32)
            nc.vector.tensor_copy(s2_sb[:], ps_s2[0:1, :N])
            ps_s2b = small_ps.tile([P, VB], F32, tag="sm")
            nc.tensor.matmul(ps_s2b[:, :N], ones_n[:], s2_sb[:], start=True, stop=True)
            s2b_sb = memp.tile([P, N], F32)
            nc.vector.tensor_copy(s2b_sb[:], ps_s2b[:, :N])

            outs_sb = memp.tile([P, 2, N], F32)
            for m in range(2):
                a = memp.tile([P, N], F32, tag="ma")
                nc.vector.tensor_tensor(a[:], er_sb[:, m, :], ps_t12[:, ds((2 + m) * N, N)],
                                        op=ALU.mult)
                bb = memp.tile([P, N], F32, tag="mb")
                nc.vector.tensor_tensor(bb[:], up_sb[:, m, :], s2b_sb[:], op=ALU.mult)
                cc = memp.tile([P, N], F32, tag="mc")
                nc.vector.scalar_tensor_tensor(cc[:], a[:], -1.0, ps_t12[:, ds(m * N, N)],
                                               op0=ALU.mult, op1=ALU.add)
                nc.vector.tensor_tensor(outs_sb[:, m, :], cc[:], bb[:], op=ALU.add)
                nc.sync.dma_start(d_outsT[m, :, :], outs_sb[:, m, :])

        # =================================================================
        # Phase F: decoder -> recon
        # =================================================================
        with tc.tile_pool(name="dec_io", bufs=3) as dec_io:
            for c in range(N_DCB):
                c0 = c * DCB
                cw = min(DCB, V - c0)
                xw_ = dec_io.tile([D, DCB], F32, tag="xd")
                nc.sync.dma_start(xw_[:, :cw], d_xdecWT[:, ds(c0, cw)])
                cw_ = dec_io.tile([K, DCB], F32, tag="cd")
                nc.sync.dma_start(cw_[:, :cw], d_ctxdecWT[:, ds(c0, cw)])
                bz = dec_io.tile([1, DCB], F32, tag="bz")
                nc.sync.dma_start(bz[:, :cw], d_decb[0:1, ds(c0, cw)])
                ps = small_ps.tile([P, DCB], F32, tag=("sm" if c % 2 else "t12"),
                                   name="ps_dec")
                nc.tensor.matmul(ps[:, :cw], sdT[:, :], xw_[:, :cw], start=True, stop=False)
                nc.tensor.matmul(ps[:, :cw], softtT[:, :], cw_[:, :cw], start=False, stop=False)
                nc.tensor.matmul(ps[:, :cw], ones_n[:], bz[:, :cw], start=False, stop=True)
                rec = dec_io.tile([P, DCB], F32, tag="rec")
                nc.scalar.copy(rec[:, :cw], ps[:, :cw])
                nc.sync.dma_start(d_recon[:, ds(c0, cw)], rec[:, :cw])

    nc.compile()
    return nc


# --------------------------------------------------------------------------
# host glue
# --------------------------------------------------------------------------

_PROGRAM = None


def _get_program():
    global _PROGRAM
    if _PROGRAM is None:
        _PROGRAM = _build_program()
    return _PROGRAM


def _prep_inputs(inputs):
    f = lambda x: np.ascontiguousarray(np.asarray(x, dtype=np.float32))
    word_id = np.asarray(inputs["word_id"]).astype(np.int32)
    sent_lens = np.asarray(inputs["sent_lens"]).astype(np.int64)
    w_ih, w_hh = f(inputs["w_ih"]), f(inputs["w_hh"])
    b_ih, b_hh = f(inputs["b_ih"]), f(inputs["b_hh"])

    # wihT [128, 3, 1536]: chunk c rows = w_ih.T rows c*128...; chunk 2 row 44
    # carries the folded bias (b_ih+b_hh for r/z columns, b_ih for n columns).
    wihT = np.zeros((P, 3, 3 * H), np.float32)
    wT = w_ih.T  # [300, 1536]
    wihT[:, 0, :] = wT[0:128]
    wihT[:, 1, :] = wT[128:256]
    wihT[0:44, 2, :] = wT[256:300]
    folded = np.concatenate([b_ih[: 2 * H] + b_hh[: 2 * H], b_ih[2 * H:]])
    wihT[44, 2, :] = folded
    whhT = np.ascontiguousarray(w_hh.T.reshape(4, P, 3 * H).transpose(1, 0, 2))

    attnWT = np.ascontiguousarray(f(inputs["attn_W"]).T.reshape(4, P, H).transpose(1, 0, 2))
    attn_b_col = np.ascontiguousarray(f(inputs["attn_b"]).reshape(4, P).T)
    attn_v_col = np.ascontiguousarray(f(inputs["attn_v"]).reshape(4, P).T)

    gumbel, eps = _sampling_noise()

    mask = np.where(
        np.arange(NW)[:, None] < sent_lens[None, :], np.float32(0.0), np.float32(-1e9)
    ).astype(np.float32)  # [NW, NS]

    zmuWT = np.ascontiguousarray(f(inputs["zmu_W"]).T.reshape(4, P, D).transpose(1, 0, 2))
    zlvWT = np.ascontiguousarray(f(inputs["zlv_W"]).T.reshape(4, P, D).transpose(1, 0, 2))
    eraserWT = np.ascontiguousarray(f(inputs["eraser_W"]).T.reshape(4, P, M).transpose(1, 0, 2))
    updateWT = np.ascontiguousarray(f(inputs["update_W"]).T.reshape(4, P, M).transpose(1, 0, 2))

    shared = {
        "emb_tab": f(inputs["word_emb"]),
        "wihT": wihT,
        "whhT": whhT,
        "b_hn_row": b_hh[2 * H:].reshape(1, H),
        "attnWT": attnWT,
        "attn_b_col": attn_b_col,
        "attn_v_col": attn_v_col,
        "xencWT": np.ascontiguousarray(f(inputs["xenc_W"]).T),
        "xenc_b_row": f(inputs["xenc_b"]).reshape(1, D),
        "ctxencWT": np.ascontiguousarray(f(inputs["ctxenc_W"]).T),
        "ctxenc_b_row": f(inputs["ctxenc_b"]).reshape(1, H),
        "ctxenc_b_blk": f(inputs["ctxenc_b"]).reshape(4, P),
        "indicator": np.kron(np.eye(4, dtype=np.float32), np.ones((1, P), np.float32)),
        "zmuWT": zmuWT,
        "zmu_b_row": f(inputs["zmu_b"]).reshape(1, D),
        "zlvWT": zlvWT,
        "zlv_b_row": f(inputs["zlv_b"]).reshape(1, D),
        "xdecWT": np.ascontiguousarray(f(inputs["xdec_W"]).T),
        "ctxdecWT": np.ascontiguousarray(f(inputs["ctxdec_W"]).T),
        "dec_b_row": (f(inputs["xdec_b"]) + f(inputs["ctxdec_b"])).reshape(1, V),
        "eraserWT": eraserWT,
        "eraser_b_row": f(inputs["eraser_b"]).reshape(1, M),
        "updateWT": updateWT,
        "update_b_row": f(inputs["update_b"]).reshape(1, M),
        "memW": f(inputs["memory"]),
    }

    bow = f(inputs["sent_bow"])
    in_maps = []
    for c in range(NCORES):
        sl = slice(c * N, (c + 1) * N)
        m = dict(shared)
        m["wid"] = np.ascontiguousarray(word_id[sl])
        m["bow"] = np.ascontiguousarray(bow[sl])
        m["addmask"] = np.ascontiguousarray(mask[:, sl].reshape(1, NW * N))
        m["gumbel"] = np.ascontiguousarray(
            gumbel[:, sl, :].transpose(1, 0, 2).reshape(N, DS * D)
        )
        m["epsT"] = np.ascontiguousarray(eps[sl].T)
        in_maps.append(m)
    return in_maps


def kernel(**inputs):
    nc = _get_program()
    in_maps = _prep_inputs(inputs)
    res = run_bass_kernel_spmd(nc, in_maps, list(range(NCORES)))
    recon = np.concatenate([r["recon"] for r in res.results], axis=0)
    outs = np.concatenate(
        [r["outsT"].transpose(2, 0, 1).reshape(N, M) for r in res.results], axis=0
    )
    turn_lens = np.asarray(inputs["turn_lens"]).astype(np.int64)
    last_idx = np.cumsum(turn_lens) - 1
    fc_W = np.asarray(inputs["fc_W"], dtype=np.float32)
    score = outs[last_idx] @ fc_W.T
    return recon.astype(np.float32), score.astype(np.float32)
